# revision 14
# baseline (speedup 1.0000x reference)
"""Graycode encoder kernel for Trainium2 (Bass/Tile), 8-core data-parallel.

Input  X: (8, 65536, 3) float32 (full).
Output:   (8, 65536, 96) int32 (full).

Per coordinate dim d (each 32 output channels):
  raw  = round(x)            (RNE, matches jnp.round)
  sign = raw > 0             -> channel 32*d
  g    = |raw| ^ (|raw| >> 1)
  bit k of g (k=0..30)       -> channel 32*d + 1 + k

The 32 channels for one (point, dim) are bits of one packed word, so the
device emits a packed (65536, 3) tensor (int16 fast path / int32 exact
fallback) instead of the unpacked (65536, 96) int32 -- 64x / 32x less HBM
write traffic -- and the host expands with np.unpackbits.

Fast path (int16, valid when round(|x|) < 2^14):
  w16 = g | (sign << 15)     (bit 15 = sign, bits 0..13 = gray bits 0..13,
                              bit 14 dead; channels 15..31 of the dim are 0)
computed as:
  ACT : absi = int16(|x|)                     (f32->i16 RNE convert)
  DVE : s15  = (x > 0.5) * -32768  -> int16   (exact 0x8000 / 0x0000;
        two-scalar tensor_scalar, f32 reads run the 2x perf mode)
  DVE : G    = (P >> 1) ^ P   on int32 PAIRS  (P = absi bitcast to i32;
        halves the element count -- i16 scalar_tensor_tensor has no fast
        uop, so pair-packing is the only 2x available to it.  The shift
        leaks bit0 of the odd lane into bit 15 of the even lane; that bit
        is dead and masked in the next op.)
  DVE : W    = (G & 0x3FFF3FFF) | S15pairs    (mask kills the leak + dead
        bit 14, OR injects both lanes' sign bits)
Fallback (int32): w32 = (g << 1) | sign, host unpacks 32 bits/dim.

Sharding: batch axis across the 8 cores (core b handles X[b]).

Per-core layout: points n = p*512 + t with p in [0,128) the SBUF partition
and t in [0,512).  t-chunks sized small-first so compute starts as soon as
the first slice of the input stream lands.  All input DMAs issue from Sync
(HWDGE, back-to-back thanks to pin bufs=4, one continuous ~220GB/s stream);
output DMAs alternate GpSimd/Sync so the last chunk's descriptor
generation never queues behind the previous chunk's.  The bufs=1 pool for
s15/g forces the Tile scheduler to order the DVE stream chunk-by-chunk
(without it, later chunks' input-gated ops get hoisted ahead of
compute-gated ones and the DVE idles ~1.5us waiting for DMA data).
"""

import numpy as np

import concourse.tile as tile
from concourse import bacc, mybir
from concourse.bass_utils import run_bass_kernel_spmd

A = mybir.AluOpType
ACTF = mybir.ActivationFunctionType
F32, I32, I16 = mybir.dt.float32, mybir.dt.int32, mybir.dt.int16

B, N, D = 8, 65536, 3
P = 128            # SBUF partitions
T = N // P         # 512 t-values per partition
CHUNKS = (48, 144, 160, 160)
MASK = 0x3FFF3FFF

_CACHE = {}


def _stt_int(eng, out, in0, scalar, in1, op0, op1):
    """scalar_tensor_tensor with an int immediate matching in0's dtype:
    out = (in0 op0 s) op1 in1."""
    return eng.add_instruction(
        mybir.InstTensorScalarPtr(
            name=eng.bass.get_next_instruction_name(),
            is_scalar_tensor_tensor=True,
            op0=op0,
            op1=op1,
            ins=[eng.lower_ap(in0),
                 mybir.ImmediateValue(dtype=in0.tensor.dtype, value=scalar),
                 eng.lower_ap(in1)],
            outs=[eng.lower_ap(out)],
        )
    )


def _build(use_i16):
    key = "i16" if use_i16 else "i32"
    if key in _CACHE:
        return _CACHE[key]

    OT = I16 if use_i16 else I32
    maxc = max(CHUNKS)

    nc = bacc.Bacc("TRN2", target_bir_lowering=False, debug=False, num_devices=B)
    x = nc.dram_tensor("x", [N, D], F32, kind="ExternalInput").ap()
    out = nc.dram_tensor("out", [N, D], OT, kind="ExternalOutput").ap()

    x_r = x.rearrange("(p t) d -> p (t d)", p=P)      # [128, 1536] f32
    out_r = out.rearrange("(p t) d -> p (t d)", p=P)  # [128, 1536] OT

    with tile.TileContext(nc) as tc:
        with (
            tc.tile_pool(name="pin", bufs=4) as pin,
            tc.tile_pool(name="pabs", bufs=2) as pabs,
            # bufs=1: the WAR dep through buffer recycling forces the Tile
            # scheduler to order DVE chunk-by-chunk (chunk c+1's s15/g wait
            # on chunk c's W) instead of hoisting later chunks' data-ready
            # ops ahead of compute-ready ones.
            tc.tile_pool(name="pser", bufs=1) as pser,
            tc.tile_pool(name="pout", bufs=4) as pout,
        ):
            t0 = 0
            for ci, c in enumerate(CHUNKS):
                w = c * D
                tin_full = pin.tile([P, maxc * D], F32, tag="tin")
                tin = tin_full[:, :w]
                # all inputs on the Sync HWDGE ring: sequential streaming
                # there still beats GpSimd's SWDGE ~1us first-byte latency
                nc.sync.dma_start(tin, x_r[:, t0 * D:t0 * D + w])

                # absi = int(round(|x|)) on ACT (RNE output conversion)
                absi_full = pabs.tile([P, maxc * D], OT, tag="absi")
                absi = absi_full[:, :w]
                nc.scalar.activation(absi, tin, ACTF.Abs)

                wout_full = pout.tile([P, maxc * D], OT, tag="w")
                wout = wout_full[:, :w]

                if use_i16:
                    # s15 = (x > 0.5) * -32768 : exact 0x8000/0x0000 int16
                    s15_full = pser.tile([P, maxc * D], I16, tag="s15")
                    s15 = s15_full[:, :w]
                    nc.vector.tensor_scalar(s15, tin, 0.5, -32768.0,
                                            A.is_gt, A.mult)

                    # int32-pair views (halves DVE element count)
                    pP = absi.bitcast(I32)
                    g_full = pser.tile([P, maxc * D // 2], I32, tag="g")
                    g = g_full[:, :w // 2]
                    _stt_int(nc.vector, g, pP, 1, pP,
                             A.logical_shift_right, A.bitwise_xor)
                    _stt_int(nc.vector, wout.bitcast(I32), g, MASK,
                             s15.bitcast(I32), A.bitwise_and, A.bitwise_or)
                else:
                    # sign channel: round(x) > 0  <=>  x > 0.5
                    sgn_full = pser.tile([P, maxc * D], I32, tag="sgn")
                    sgn = sgn_full[:, :w]
                    nc.vector.tensor_scalar(sgn, tin, 0.5, None, A.is_gt)

                    g_full = pser.tile([P, maxc * D], I32, tag="g")
                    g = g_full[:, :w]
                    _stt_int(nc.vector, g, absi, 1, absi,
                             A.logical_shift_right, A.bitwise_xor)
                    _stt_int(nc.vector, wout, g, 1, sgn,
                             A.logical_shift_left, A.bitwise_or)

                # alternate output rings so the last chunk's descriptor
                # generation never queues behind the previous chunk's
                out_eng = nc.gpsimd if ci % 2 == 0 else nc.sync
                out_eng.dma_start(out_r[:, t0 * D:t0 * D + w], wout)
                t0 += c

    nc.compile()
    _CACHE[key] = nc
    return nc


def kernel(X, **run_kwargs):
    X = np.asarray(X, dtype=np.float32)
    assert X.shape == (B, N, D), X.shape
    use_i16 = bool(np.abs(X).max() < 16383.49)
    nc = _build(use_i16)
    in_maps = [{"x": np.ascontiguousarray(X[b])} for b in range(B)]
    res = run_bass_kernel_spmd(nc, in_maps, core_ids=list(range(B)), **run_kwargs)
    w = np.stack([r["out"] for r in res.results], axis=0)   # (B, N, D) i16/i32
    if use_i16:
        by = np.ascontiguousarray(w).view(np.uint8).reshape(B, N, D * 2)
        bits = np.unpackbits(by, axis=-1, bitorder="little").reshape(B, N, D, 16)
        out = np.zeros((B, N, D, 32), dtype=np.int32)
        out[..., 0] = bits[..., 15]          # sign (bit 15)
        out[..., 1:15] = bits[..., 0:14]     # gray bits 0..13
        out = out.reshape(B, N, D * 32)
    else:
        by = np.ascontiguousarray(w).view(np.uint8).reshape(B, N, D * 4)
        out = np.unpackbits(by, axis=-1, bitorder="little").astype(np.int32)
    if run_kwargs:
        kernel.last_result = res
    return out


# revision 15
# speedup vs baseline: 1.0283x; 1.0283x over previous
"""Graycode encoder kernel for Trainium2 (Bass/Tile), 8-core data-parallel.

Input  X: (8, 65536, 3) float32 (full).
Output:   (8, 65536, 96) int32 (full).

Per coordinate dim d (each 32 output channels):
  raw  = round(x)            (RNE, matches jnp.round)
  sign = raw > 0             -> channel 32*d
  g    = |raw| ^ (|raw| >> 1)
  bit k of g (k=0..30)       -> channel 32*d + 1 + k

The 32 channels for one (point, dim) are bits of one packed word, so the
device emits a packed (65536, 3) tensor (int16 fast path / int32 exact
fallback) instead of the unpacked (65536, 96) int32 -- 64x / 32x less HBM
write traffic -- and the host expands with np.unpackbits.

Fast path (int16, valid when round(|x|) < 2^14):
  w16 = g | (sign << 15)     (bit 15 = sign, bits 0..13 = gray bits 0..13,
                              bit 14 dead; channels 15..31 of the dim are 0)
computed as:
  ACT : absi = int16(|x|)                     (f32->i16 RNE convert)
  DVE : s15  = (x > 0.5) * -32768  -> int16   (exact 0x8000 / 0x0000;
        two-scalar tensor_scalar, f32 reads run the 2x perf mode)
  DVE : G    = (P >> 1) ^ P   on int32 PAIRS  (P = absi bitcast to i32;
        halves the element count -- i16 scalar_tensor_tensor has no fast
        uop, so pair-packing is the only 2x available to it.  The shift
        leaks bit0 of the odd lane into bit 15 of the even lane; that bit
        is dead and masked in the next op.)
  DVE : W    = (G & 0x3FFF3FFF) | S15pairs    (mask kills the leak + dead
        bit 14, OR injects both lanes' sign bits)
Fallback (int32): w32 = (g << 1) | sign, host unpacks 32 bits/dim.

Sharding: batch axis across the 8 cores (core b handles X[b]).

Per-core layout: points n = p*512 + t with p in [0,128) the SBUF partition
and t in [0,512).  t-chunks sized small-first so compute starts as soon as
the first slice of the input stream lands.  All input DMAs issue from Sync
(HWDGE, back-to-back thanks to pin bufs=4, one continuous ~220GB/s stream);
output DMAs alternate GpSimd/Sync so the last chunk's descriptor
generation never queues behind the previous chunk's.  The bufs=1 pool for
s15/g forces the Tile scheduler to order the DVE stream chunk-by-chunk
(without it, later chunks' input-gated ops get hoisted ahead of
compute-gated ones and the DVE idles ~1.5us waiting for DMA data).
"""

import numpy as np

import concourse.tile as tile
from concourse import bacc, mybir
from concourse.bass_utils import run_bass_kernel_spmd

A = mybir.AluOpType
ACTF = mybir.ActivationFunctionType
F32, I32, I16 = mybir.dt.float32, mybir.dt.int32, mybir.dt.int16

B, N, D = 8, 65536, 3
P = 128            # SBUF partitions
T = N // P         # 512 t-values per partition
CHUNKS = (64, 128, 160, 160)
MASK = 0x3FFF3FFF

_CACHE = {}


def _stt_int(eng, out, in0, scalar, in1, op0, op1):
    """scalar_tensor_tensor with an int immediate matching in0's dtype:
    out = (in0 op0 s) op1 in1."""
    return eng.add_instruction(
        mybir.InstTensorScalarPtr(
            name=eng.bass.get_next_instruction_name(),
            is_scalar_tensor_tensor=True,
            op0=op0,
            op1=op1,
            ins=[eng.lower_ap(in0),
                 mybir.ImmediateValue(dtype=in0.tensor.dtype, value=scalar),
                 eng.lower_ap(in1)],
            outs=[eng.lower_ap(out)],
        )
    )


def _build(use_i16):
    key = "i16" if use_i16 else "i32"
    if key in _CACHE:
        return _CACHE[key]

    OT = I16 if use_i16 else I32
    maxc = max(CHUNKS)

    nc = bacc.Bacc("TRN2", target_bir_lowering=False, debug=False, num_devices=B)
    x = nc.dram_tensor("x", [N, D], F32, kind="ExternalInput").ap()
    out = nc.dram_tensor("out", [N, D], OT, kind="ExternalOutput").ap()

    x_r = x.rearrange("(p t) d -> p (t d)", p=P)      # [128, 1536] f32
    out_r = out.rearrange("(p t) d -> p (t d)", p=P)  # [128, 1536] OT

    with tile.TileContext(nc) as tc:
        with (
            tc.tile_pool(name="pin", bufs=4) as pin,
            tc.tile_pool(name="pabs", bufs=2) as pabs,
            # bufs=1: the WAR dep through buffer recycling forces the Tile
            # scheduler to order DVE chunk-by-chunk (chunk c+1's s15/g wait
            # on chunk c's W) instead of hoisting later chunks' data-ready
            # ops ahead of compute-ready ones.
            tc.tile_pool(name="pser", bufs=1) as pser,
            tc.tile_pool(name="pout", bufs=4) as pout,
        ):
            t0 = 0
            for ci, c in enumerate(CHUNKS):
                w = c * D
                tin_full = pin.tile([P, maxc * D], F32, tag="tin")
                tin = tin_full[:, :w]
                # all inputs on the Sync HWDGE ring: sequential streaming
                # there still beats GpSimd's SWDGE ~1us first-byte latency
                nc.sync.dma_start(tin, x_r[:, t0 * D:t0 * D + w])

                # absi = int(round(|x|)) on ACT (RNE output conversion)
                absi_full = pabs.tile([P, maxc * D], OT, tag="absi")
                absi = absi_full[:, :w]
                nc.scalar.activation(absi, tin, ACTF.Abs)

                wout_full = pout.tile([P, maxc * D], OT, tag="w")
                wout = wout_full[:, :w]

                if use_i16:
                    # s15 = (x > 0.5) * -32768 : exact 0x8000/0x0000 int16
                    s15_full = pser.tile([P, maxc * D], I16, tag="s15")
                    s15 = s15_full[:, :w]
                    nc.vector.tensor_scalar(s15, tin, 0.5, -32768.0,
                                            A.is_gt, A.mult)

                    # int32-pair views (halves DVE element count)
                    pP = absi.bitcast(I32)
                    g_full = pser.tile([P, maxc * D // 2], I32, tag="g")
                    g = g_full[:, :w // 2]
                    _stt_int(nc.vector, g, pP, 1, pP,
                             A.logical_shift_right, A.bitwise_xor)
                    _stt_int(nc.vector, wout.bitcast(I32), g, MASK,
                             s15.bitcast(I32), A.bitwise_and, A.bitwise_or)
                else:
                    # sign channel: round(x) > 0  <=>  x > 0.5
                    sgn_full = pser.tile([P, maxc * D], I32, tag="sgn")
                    sgn = sgn_full[:, :w]
                    nc.vector.tensor_scalar(sgn, tin, 0.5, None, A.is_gt)

                    g_full = pser.tile([P, maxc * D], I32, tag="g")
                    g = g_full[:, :w]
                    _stt_int(nc.vector, g, absi, 1, absi,
                             A.logical_shift_right, A.bitwise_xor)
                    _stt_int(nc.vector, wout, g, 1, sgn,
                             A.logical_shift_left, A.bitwise_or)

                # alternate output rings so the last chunk's descriptor
                # generation never queues behind the previous chunk's
                out_eng = nc.gpsimd if ci % 2 == 0 else nc.sync
                out_eng.dma_start(out_r[:, t0 * D:t0 * D + w], wout)
                t0 += c

    nc.compile()
    _CACHE[key] = nc
    return nc


def kernel(X, **run_kwargs):
    X = np.asarray(X, dtype=np.float32)
    assert X.shape == (B, N, D), X.shape
    use_i16 = bool(np.abs(X).max() < 16383.49)
    nc = _build(use_i16)
    in_maps = [{"x": np.ascontiguousarray(X[b])} for b in range(B)]
    res = run_bass_kernel_spmd(nc, in_maps, core_ids=list(range(B)), **run_kwargs)
    w = np.stack([r["out"] for r in res.results], axis=0)   # (B, N, D) i16/i32
    if use_i16:
        by = np.ascontiguousarray(w).view(np.uint8).reshape(B, N, D * 2)
        bits = np.unpackbits(by, axis=-1, bitorder="little").reshape(B, N, D, 16)
        out = np.zeros((B, N, D, 32), dtype=np.int32)
        out[..., 0] = bits[..., 15]          # sign (bit 15)
        out[..., 1:15] = bits[..., 0:14]     # gray bits 0..13
        out = out.reshape(B, N, D * 32)
    else:
        by = np.ascontiguousarray(w).view(np.uint8).reshape(B, N, D * 4)
        out = np.unpackbits(by, axis=-1, bitorder="little").astype(np.int32)
    if run_kwargs:
        kernel.last_result = res
    return out
